# revision 16
# baseline (speedup 1.0000x reference)
"""Trainium2 Bass kernel for nn_Attention_78786880078278.

Dense causal multi-head attention layer (QKV proj + RoPE + causal softmax
attention + output proj), sharded over 8 NeuronCores:
  - NEFF 1 (head-parallel): each core computes QKV projections, RoPE and
    causal attention for its 2 heads (x 2 batches), producing per-head
    attention outputs.
  - host: pure relayout (gather + transpose) of the per-head outputs.
  - NEFF 2 (token-parallel): each core computes the output projection for
    its 512-token slice.

All matmuls run in bf16 with fp32 PSUM accumulation. Inputs are cast to
bf16 on the host (pure dtype marshaling) so the device never runs f32
casts and x DMA traffic is halved; all FLOPs run on device.
"""

import contextlib
import ctypes
import hashlib
import json
import math
import os
import shutil
import sys
import types

import numpy as np
import ml_dtypes

# ---------------------------------------------------------------------------
# environment fixups
# ---------------------------------------------------------------------------

for _p in ("/opt/trn_rl_repo",):
    if _p not in sys.path and os.path.isdir(_p):
        sys.path.append(_p)

import concourse.bass as bass  # noqa: E402
import concourse.bass_isa as bass_isa  # noqa: E402
import concourse.bass2jax as bass2jax  # noqa: E402
import concourse.mybir as mybir  # noqa: E402
import concourse.tile as tile  # noqa: E402
from concourse.bass_utils import run_bass_kernel_spmd  # noqa: E402

F32 = mybir.dt.float32
BF16 = mybir.dt.bfloat16
NPBF16 = ml_dtypes.bfloat16

_NEFF_CACHE_DIR = os.environ.get("NEFF_CACHE_DIR", "/tmp/neff_cache")


def _install_compile_fixups():
    """(1) Split multi-wait instructions: this walrus build encodes a single
    sync-wait slot per instruction and rejects Tile's final multi-wait drain.
    (2) Cache compiled NEFFs by BIR hash so repeated runs skip walrus."""
    if getattr(bass2jax, "_attn_fixup_installed", False):
        return
    orig = bass2jax.compile_bir_kernel

    def _fix_multiwait(bir_bytes):
        bir = json.loads(bir_bytes)
        changed = False
        for fn in bir.get("functions", []):
            for blk in fn.get("basic_blocks", fn.get("blocks", [])):
                # Drop a Ldweights whose operand AP matches the previous
                # Ldweights on this engine queue and which carries no sync
                # info: Tile attaches a wait to the instruction whenever its
                # input was rewritten, so a wait-free identical reload is
                # redundant (the weights are still in the PE array).
                pruned = []
                last_ldw = None
                for inst in blk.get("instructions", []):
                    op = inst.get("opcode")
                    if inst.get("engine") == "PE":
                        if op == "Ldweights":
                            si = inst.get("sync_info") or {}
                            key = json.dumps(
                                [inst.get("ins"), inst.get("perf_mode"),
                                 inst.get("is_transpose"),
                                 inst.get("tile_position")],
                                sort_keys=True,
                            )
                            if (last_ldw == key
                                    and not si.get("on_wait")
                                    and not si.get("on_update")):
                                changed = True
                                continue
                            last_ldw = key
                        elif op not in ("Matmult", "EventSemaphore", "Drain",
                                        "Nop", "Notify"):
                            last_ldw = None
                    pruned.append(inst)
                blk["instructions"] = pruned
                new_insts = []
                for inst in pruned:
                    si = inst.get("sync_info") or {}
                    waits = si.get("on_wait") or []
                    if len(waits) > 1:
                        changed = True
                        for i, w in enumerate(waits[:-1]):
                            pre = {
                                "name": f"{inst['name']}_w{i}",
                                "opcode": "Drain",
                                "engine": inst["engine"],
                                "ins": [],
                                "outs": [],
                                "sync_info": {"on_wait": [w], "on_update": []},
                            }
                            if "debug" in inst:
                                pre["debug"] = inst["debug"]
                            if "is_reset_sema" in inst:
                                pre["is_reset_sema"] = False
                            new_insts.append(pre)
                        si["on_wait"] = [waits[-1]]
                        inst["sync_info"] = si
                    new_insts.append(inst)
                blk["instructions"] = new_insts
        return json.dumps(bir).encode() if changed else bir_bytes

    def _patched(bir_json, tmpdir, neff_name="file.neff"):
        fixed = _fix_multiwait(bir_json)
        key = hashlib.sha256(fixed).hexdigest()[:24]
        cached = os.path.join(_NEFF_CACHE_DIR, f"{key}.neff")
        target = os.path.join(tmpdir, neff_name)
        if os.path.exists(cached):
            shutil.copy(cached, target)
            return target
        path = orig(fixed, tmpdir, neff_name)
        try:
            os.makedirs(_NEFF_CACHE_DIR, exist_ok=True)
            shutil.copy(path, cached)
        except OSError:
            pass
        return path

    bass2jax.compile_bir_kernel = _patched

    bass2jax._attn_fixup_installed = True


def _install_ntff_hook():
    """Register the NTFF profiling hook (used only when BASS_TRACE=1)."""
    try:
        import antenv
    except ImportError:
        return
    if "antenv.axon_hooks" in sys.modules:
        return
    so_path = "/opt/axon/libaxon_pjrt.so"
    try:
        lib = ctypes.CDLL(so_path)
    except OSError:
        return
    if not hasattr(lib, "axon_start_nrt_profile"):
        return
    lib.axon_start_nrt_profile.argtypes = [
        ctypes.POINTER(ctypes.c_int64),
        ctypes.c_size_t,
    ]
    lib.axon_start_nrt_profile.restype = ctypes.c_int64
    lib.axon_stop_nrt_profile.argtypes = [ctypes.c_char_p]
    lib.axon_stop_nrt_profile.restype = ctypes.c_int64

    @contextlib.contextmanager
    def _hook(output_dir, device_ids):
        import jax

        jax.devices()
        if device_ids:
            ids = (ctypes.c_int64 * len(device_ids))(*device_ids)
            rc = lib.axon_start_nrt_profile(ids, len(device_ids))
        else:
            rc = lib.axon_start_nrt_profile(None, 0)
        if rc != 0:
            raise RuntimeError(f"axon_start_nrt_profile rc={rc}")
        try:
            yield
        finally:
            n = lib.axon_stop_nrt_profile(str(output_dir).encode())
            print(f"profile: {n} file(s) in {output_dir}", file=sys.stderr)

    mod = types.ModuleType("antenv.axon_hooks")
    mod.get_axon_ntff_profile_hook = lambda: _hook
    mod.set_axon_ntff_profile_hook = lambda h: None
    sys.modules["antenv.axon_hooks"] = mod
    antenv.axon_hooks = mod


_install_compile_fixups()
_install_ntff_hook()

# ---------------------------------------------------------------------------
# problem constants (hardcoded per the task spec)
# ---------------------------------------------------------------------------

HIDDEN = 2048
HEADS = 16
HD = 128  # head dim
B = 2
S = 2048
N_CORES = 8
HPC = HEADS // N_CORES  # heads per core = 2
SPAN = 512
NSPANS = S // SPAN  # 4 query spans per batch
KT = HIDDEN // 128  # 16 contraction tiles
TT = S // 128  # 16 token tiles per batch
SCALE = 1.0 / math.sqrt(HD)
TOK_SLICE = (B * S) // N_CORES  # 512 tokens per core in NEFF 2

LAST_RESULTS = []  # BassKernelResults of the most recent kernel() call


# ---------------------------------------------------------------------------
# NEFF 1: QKV projections + RoPE + causal attention for 2 heads x 2 batches
# ---------------------------------------------------------------------------

def build_attn_nc():
    """QKV + RoPE + causal attention for 2 heads x 2 batches per core.

    All inputs arrive bf16 (host-marshaled), so x slabs DMA straight into
    the pair buffers and weights/tables DMA straight into their persistent
    tiles -- no device-side casts.

    Structure (all phases software-pipelined):
      - QKV: q/k weight-stationary (psum [hd, tok]), v x-stationary
        (psum [tok, hd]).
      - RoPE in bf16 (psum -> bf16 copy, two half-rotate DMAs, 3 DVE ops).
      - Attention per (b,h) in two half-passes (spans 01 then 23), kt-major:
        one k-tile LDWEIGHTS serves all valid spans' score matmuls; exp on
        ScalarE; AV is v-stationary streaming the exp tiles (512-wide);
        diagonal k-tiles stream only the causal suffix (128-granular).
      - Softmax denominator: ones-stationary matmul re-stream of the exp
        tiles; 1/den via exp(-ln(den)) on ScalarE.
      - b1 QKV work is interleaved into the four attention blocks to hide
        exp latency and keep the PE warm; the last block's first half is
        filled with b1h1's pair-1 q/k passes.
    Output: attnout[h, b, hd, s] bf16 (head-major for the host relayout).
    """
    nc = bass.Bass(target_bir_lowering=False, debug=False)

    xT = nc.dram_tensor("xT", [B, HIDDEN, S], BF16, kind="ExternalInput")
    wqT = nc.dram_tensor("wqT", [HIDDEN, HPC * HD], BF16, kind="ExternalInput")
    wkT = nc.dram_tensor("wkT", [HIDDEN, HPC * HD], BF16, kind="ExternalInput")
    wvT = nc.dram_tensor("wvT", [HIDDEN, HPC * HD], BF16, kind="ExternalInput")
    cosT = nc.dram_tensor("cosT", [HD, S], BF16, kind="ExternalInput")
    sinT = nc.dram_tensor("sinT", [HD, S], BF16, kind="ExternalInput")  # sign-folded
    maskd = nc.dram_tensor("mask", [128, 128], BF16, kind="ExternalInput")
    attnout = nc.dram_tensor(
        "attnout", [HPC, B, NSPANS, 128, 4, HD], BF16, kind="ExternalOutput"
    )
    dbg = bool(os.environ.get("BASS_DEBUG_DUMP"))
    if dbg:
        qdump = nc.dram_tensor("qdump", [HD, B, HPC, S], BF16,
                               kind="ExternalOutput")
        kdump = nc.dram_tensor("kdump", [HD, B, HPC, S], BF16,
                               kind="ExternalOutput")
        vdump = nc.dram_tensor("vdump", [128, B, TT, HPC * HD], BF16,
                               kind="ExternalOutput")
        ddump = nc.dram_tensor("ddump", [B, HPC, NSPANS, SPAN], F32,
                               kind="ExternalOutput")

    with tile.TileContext(nc) as tc:
        with (
            tc.tile_pool(name="persist", bufs=1) as persist,
            tc.tile_pool(name="xpool", bufs=2) as xpool,
            tc.tile_pool(name="ropep", bufs=6) as ropep,
            tc.tile_pool(name="espool", bufs=6) as espool,
            tc.tile_pool(name="normp", bufs=2) as normp,
            tc.tile_pool(name="ps_o", bufs=1, space="PSUM") as ps_o,
            tc.tile_pool(name="ps_sc", bufs=3, space="PSUM") as ps_sc,
            tc.tile_pool(name="ps_qkv", bufs=2, space="PSUM") as ps_qkv,
            tc.tile_pool(name="ps_den", bufs=1, space="PSUM") as ps_den,
        ):
            # ---------------- persistent tiles ----------------
            wq_bf = persist.tile([128, KT, HPC * HD], BF16, tag="wq_bf")
            wk_bf = persist.tile([128, KT, HPC * HD], BF16, tag="wk_bf")
            wv_bf = persist.tile([128, KT, HPC * HD], BF16, tag="wv_bf")
            cos_bf = persist.tile([HD, S], BF16, tag="cos_bf")
            sin_bf = persist.tile([HD, S], BF16, tag="sin_bf")
            mask_bf = persist.tile([128, 128], BF16, tag="mask_bf")
            q_sb = persist.tile([HD, B, HPC, S], BF16, tag="q_sb")
            k_sb = persist.tile([HD, B, HPC, S], BF16, tag="k_sb")
            v_sb = persist.tile([128, B, TT, HPC * HD], BF16, tag="v_sb")
            ones_bf = persist.tile([128, 128], BF16, tag="ones_bf")

            # ---------------- load helpers (all straight bf16 DMAs) --------
            def load_weight_piece(wdram, wbf, p):
                # piece p covers hidden rows 512p..512p+512 = k-tiles 4p..4p+3
                # issued from the scalar sequencer so weight transfers start
                # in parallel with the sync-issued x slabs
                kstep = KT // 4
                src = wdram[p * 512:(p + 1) * 512, :]
                nc.scalar.dma_start(
                    wbf[:, p * kstep:(p + 1) * kstep, :],
                    src.rearrange("(ko p) h -> p ko h", p=128),
                )

            def load_tables():
                nc.scalar.dma_start(cos_bf[:], cosT[:])
                nc.scalar.dma_start(sin_bf[:], sinT[:])
                nc.scalar.dma_start(mask_bf[:], maskd[:])
                nc.vector.memset(ones_bf[:], 1.0)

            def load_x_slab(b, pair, kt, xtile):
                # one k-tile x one span-pair of bf16 x, straight from DRAM
                nc.sync.dma_start(
                    xtile[:, kt, :],
                    xT[b, kt * 128:(kt + 1) * 128,
                       pair * 2 * SPAN:(pair + 1) * 2 * SPAN],
                )

            # ---------------- QKV helpers ----------------
            def rope(b, h, t, span, ps):
                """psum [hd, 512] f32 -> q_sb/k_sb bf16 with RoPE applied."""
                qt = ropep.tile([128, SPAN], BF16, tag="ropet")
                nc.scalar.copy(qt[:], ps[:])
                rt = ropep.tile([128, SPAN], BF16, tag="rot")
                nc.sync.dma_start(rt[0:64, :], qt[64:128, :])
                nc.sync.dma_start(rt[64:128, :], qt[0:64, :])
                sl = slice(span * SPAN, (span + 1) * SPAN)
                nc.vector.tensor_mul(qt[:], qt[:], cos_bf[:, sl])
                nc.vector.tensor_mul(rt[:], rt[:], sin_bf[:, sl])
                dst = (q_sb if t == 0 else k_sb)[:, b, h, sl]
                nc.vector.tensor_add(dst, qt[:], rt[:])

            class XFeed:
                """Emission-order-safe x streaming: consumers call ensure(kt)
                so a matmul is never emitted before the DMA that writes its
                input slab (Tile deps follow emission order)."""

                def __init__(self, b, pair, xtile):
                    self.b, self.pair, self.xtile = b, pair, xtile
                    self.done = 0

                def ensure(self, kt):
                    while self.done <= kt and self.done < KT:
                        self.pump()

                def pump(self):
                    if self.done < KT:
                        load_x_slab(self.b, self.pair, self.done, self.xtile)
                        self.done += 1
                        return True
                    return False

            def qk_pass(b, pair, xtile, h, t, feed=None):
                """One (head, q|k) projection over a span pair; generator
                yielding every 2 k-tiles (~4 matmuls) for interleaving."""
                wbf = wq_bf if t == 0 else wk_bf
                hsl = slice(h * HD, (h + 1) * HD)
                ps0 = ps_qkv.tile([128, SPAN], F32, tag="qkv", name="qk0")
                ps1 = ps_qkv.tile([128, SPAN], F32, tag="qkv", name="qk1")
                for kt in range(KT):
                    if feed is not None:
                        feed.ensure(kt)
                    for sp, ps in ((0, ps0), (1, ps1)):
                        nc.tensor.matmul(
                            ps[:],
                            wbf[:, kt, hsl],
                            xtile[:, kt, sp * SPAN:(sp + 1) * SPAN],
                            start=(kt == 0),
                            stop=(kt == KT - 1),
                        )
                    if kt % 2 == 1:
                        yield
                rope(b, h, t, pair * 2, ps0)
                rope(b, h, t, pair * 2 + 1, ps1)
                yield

            def v_pass(b, span, xtile, xoff, feed=None):
                """V projection for one span (x-stationary, [tok, 2h*hd])."""
                psv = [
                    ps_qkv.tile([128, 2 * HPC * HD], F32, tag="qkv", name="v0"),
                    ps_qkv.tile([128, 2 * HPC * HD], F32, tag="qkv", name="v1"),
                ]
                for kt in range(KT):
                    if feed is not None:
                        feed.ensure(kt)
                    for j in range(4):
                        base = (j % 2) * (HPC * HD)
                        # two 256-col chains share one psum bank: start=True
                        # clears has_written for the WHOLE bank, so only the
                        # bank's first write may use it; the second chain's
                        # first matmul overwrites where the bit is clear.
                        nc.tensor.matmul(
                            psv[j // 2][:, base:base + HPC * HD],
                            xtile[:, kt,
                                  xoff * SPAN + j * 128:xoff * SPAN + (j + 1) * 128],
                            wv_bf[:, kt, :],
                            start=(kt == 0 and j % 2 == 0),
                            stop=(kt == KT - 1),
                            skip_group_check=True,
                        )
                    if kt % 2 == 1:
                        yield
                for jj in range(2):
                    for j2 in range(2):
                        tt = span * 4 + jj * 2 + j2
                        # psv drains gate the next pass's psum WAR: split
                        # them across Vector and Scalar so they clear fast
                        src_ap = psv[jj][:, j2 * HPC * HD:(j2 + 1) * HPC * HD]
                        if jj == 0:
                            nc.vector.tensor_copy(v_sb[:, b, tt, :], src_ap)
                        else:
                            nc.scalar.copy(v_sb[:, b, tt, :], src_ap)
                yield

            def dddump_view(b, h, s):
                return ddump[b:b + 1, h, s, 0:4]

            # ---------------- attention ----------------
            def attn_half(b, h, half, fill, extra_psc=False):
                """Causal attention for one (batch, head) half-pass (spans
                01 or 23), kt-major with 128-granular diagonal suffixes.

                AV runs es-STATIONARY: oT[q,ch] = sum_k es[k,q] v[k,ch], one
                matmul per 128-q chunk. Each es-chunk LDWEIGHTS also feeds a
                1-column ones matmul accumulating the softmax denominator,
                so the denominator costs ~nothing on the matmul pipe and
                lands as [q,1] -- normalization is then a per-partition
                tensor_scalar. `fill()` interleaves foreign PE work;
                `extra_psc` borrows the idle qkv psum banks for a deeper
                score pipeline."""
                spans = (0, 1) if half == 0 else (2, 3)
                kt_hi = 4 * spans[-1] + 4
                o_ps = {
                    s: ps_o.tile([128, 4, HD], F32, tag=f"o{s % 2}",
                                 name=f"o_{b}_{h}_{s}")
                    for s in spans
                }
                # denominators for both spans pack into one narrow tile
                den = ps_den.tile([128, 8], F32, tag="d", name=f"d_{b}_{h}_{half}")
                psc_n = [0]

                def psc_tile():
                    psc_n[0] += 1
                    if extra_psc and psc_n[0] % 2 == 0:
                        return ps_qkv.tile([128, SPAN], F32, tag="qkv",
                                           name="pscx")
                    return ps_sc.tile([128, SPAN], F32, tag="psc", name="psc")

                for kt in range(kt_hi):
                    valid = [s for s in spans if kt // 4 <= s]
                    pend = {}
                    for s in valid:
                        diag = (kt // 4 == s)
                        c0 = (kt % 4) * 128 if diag else 0
                        w = SPAN - c0
                        psc = psc_tile()
                        nc.tensor.matmul(
                            psc[:, 0:w],
                            k_sb[:, b, h, kt * 128:(kt + 1) * 128],
                            q_sb[:, b, h,
                                 s * SPAN + c0:(s + 1) * SPAN],
                            start=True,
                            stop=True,
                        )
                        es = espool.tile([128, SPAN], BF16, tag="es",
                                         name="es")
                        nc.scalar.activation(
                            es[:, 0:w], psc[:, 0:w],
                            mybir.ActivationFunctionType.Exp, scale=SCALE,
                        )
                        if diag:
                            nc.vector.tensor_mul(
                                es[:, 0:128], es[:, 0:128], mask_bf[:]
                            )
                        pend[s] = (es, c0, w)
                    fill()
                    for s in valid:
                        es, c0, w = pend[s]
                        sl = s % 2
                        for c in range(c0 // 128, 4):
                            ecol = slice(c * 128 - c0, (c + 1) * 128 - c0)
                            stop_c = (kt == 4 * s + c)
                            # start=True clears has_written for the WHOLE
                            # bank, so only each bank's first chain may use
                            # it; later chains write where the bit is clear.
                            nc.tensor.matmul(
                                o_ps[s][:, c, :],
                                es[:, ecol],
                                v_sb[:, b, kt, h * HD:(h + 1) * HD],
                                start=(kt == 0 and c == 0),
                                stop=stop_c,
                                skip_group_check=True,
                            )
                            nc.tensor.matmul(
                                den[:, sl * 4 + c:sl * 4 + c + 1],
                                es[:, ecol],
                                ones_bf[:, 0:1],
                                start=(kt == 0 and c == 0 and s == spans[0]),
                                stop=stop_c,
                                skip_group_check=True,
                            )
                    # normalize a span as soon as its last k-tile lands so
                    # the work hides under later attention steps
                    for s in valid:
                        if kt != 4 * s + 3:
                            continue
                        sl = s % 2
                        # 1/den as exp(-ln(den)) on ScalarE over the tiny
                        # [128,4] tile (den >= 1 so ln is safe)
                        lnt = normp.tile([128, 4], F32, tag="lnt",
                                         name="lnt")
                        nc.scalar.activation(
                            lnt[:], den[:, sl * 4:sl * 4 + 4],
                            mybir.ActivationFunctionType.Ln,
                        )
                        recb = normp.tile([128, 4], F32, tag="recb",
                                          name="recb")
                        nc.scalar.activation(
                            recb[:], lnt[:],
                            mybir.ActivationFunctionType.Exp, scale=-1.0,
                        )
                        if dbg:
                            nc.sync.dma_start(
                                dddump_view(b, h, s), recb[0:1, :]
                            )
                        obf = normp.tile([128, 4, HD], BF16, tag="obf",
                                         name="obf")
                        for c in range(4):
                            nc.vector.tensor_scalar_mul(
                                obf[:, c, :], o_ps[s][:, c, :],
                                recb[:, c:c + 1],
                            )
                        nc.sync.dma_start(
                            attnout[h, b, s, :, :, :], obf[:]
                        )
            # ---------------- emission schedule ----------------
            # Startup: batch-0 pair 0, all q/k passes fused kt-major (7 psum
            # banks borrowed across pools) so the PE tracks the x DMA feed.
            x_p0 = xpool.tile([128, KT, 2 * SPAN], BF16, tag="x", name="x_p0")

            def p0_loads(kt):
                if kt % 4 == 0:
                    load_weight_piece(wqT, wq_bf, kt // 4)
                    load_weight_piece(wkT, wk_bf, kt // 4)
                load_x_slab(0, 0, kt, x_p0)

            startup_ps = {}
            for i, (h, t, sp) in enumerate(
                [(h, t, sp) for h in range(HPC) for t in range(2)
                 for sp in range(2)]
            ):
                pool, tag = [
                    (ps_o, "o0"), (ps_o, "o1"),
                    (ps_den, "d"), (ps_sc, "psc"),
                    (ps_sc, "psc"), (ps_sc, "psc"),
                    (ps_qkv, "qkv"), (ps_qkv, "qkv"),
                ][i]
                startup_ps[(h, t, sp)] = pool.tile(
                    [128, SPAN], F32, tag=tag, name=f"su{i}"
                )
            # prefetch 2 k-tiles ahead so the first matmul only waits on
            # slab 0 + weight piece 0
            for kt in range(2):
                p0_loads(kt)
            for kt in range(KT):
                if kt + 2 < KT:
                    p0_loads(kt + 2)
                for (h, t, sp), ps in startup_ps.items():
                    wbf = wq_bf if t == 0 else wk_bf
                    nc.tensor.matmul(
                        ps[:],
                        wbf[:, kt, h * HD:(h + 1) * HD],
                        x_p0[:, kt, sp * SPAN:(sp + 1) * SPAN],
                        start=(kt == 0),
                        stop=(kt == KT - 1),
                    )
            load_tables()
            for p in range(4):
                load_weight_piece(wvT, wv_bf, p)
            # Free the qkv-tagged startup banks FIRST: the chain below
            # re-uses them immediately, so their ropes must not queue behind
            # the other six (that ordering put a ~7.5us bubble on the PE).
            rope(0, 1, 1, 0, startup_ps[(1, 1, 0)])
            rope(0, 1, 1, 1, startup_ps[(1, 1, 1)])
            pending_ropes = [
                ((0, 1, 1), startup_ps[(0, 1, 1)]),  # psc banks
                ((1, 0, 0), startup_ps[(1, 0, 0)]),
                ((1, 0, 1), startup_ps[(1, 0, 1)]),
                ((0, 0, 0), startup_ps[(0, 0, 0)]),  # o banks
                ((0, 0, 1), startup_ps[(0, 0, 1)]),
                ((0, 1, 0), startup_ps[(0, 1, 0)]),  # den bank
            ]

            # pair 1 of batch 0: the x_p1 DMA stream interleaves with a
            # STRICTLY SEQUENTIAL chain of PE passes (passes sharing the qkv
            # psum pool must not overlap, or accumulations collide on banks).
            # The remaining startup ropes are spread one per chain tick.
            import collections as _c
            x_p1 = xpool.tile([128, KT, 2 * SPAN], BF16, tag="x", name="x_p1")
            feed1 = XFeed(0, 1, x_p1)
            pe_chain = _c.deque([
                v_pass(0, 0, x_p0, 0),
                v_pass(0, 1, x_p0, 1),
                qk_pass(0, 1, x_p1, 0, 0, feed=feed1),
                qk_pass(0, 1, x_p1, 0, 1, feed=feed1),
                v_pass(0, 2, x_p1, 0, feed=feed1),
                qk_pass(0, 1, x_p1, 1, 0, feed=feed1),
                v_pass(0, 3, x_p1, 1, feed=feed1),
                qk_pass(0, 1, x_p1, 1, 1, feed=feed1),
            ])
            feeding = True
            while pe_chain or feeding:
                if pending_ropes:
                    (hh, tt_, ss), ps = pending_ropes.pop(0)
                    rope(0, hh, tt_, ss, ps)
                feeding = feed1.pump()
                n = 2
                while n and pe_chain:
                    try:
                        next(pe_chain[0])
                        n -= 1
                    except StopIteration:
                        pe_chain.popleft()

            # b1 x buffers (reuse the two pair buffers)
            x_p2 = xpool.tile([128, KT, 2 * SPAN], BF16, tag="x", name="x_p2")
            x_p3 = xpool.tile([128, KT, 2 * SPAN], BF16, tag="x", name="x_p3")

            def make_fill(gens, feed=None):
                dq = _c.deque(gens)

                def fill():
                    if feed is not None:
                        feed.pump()
                    n = 2
                    while n and dq:
                        try:
                            next(dq[0])
                            n -= 1
                        except StopIteration:
                            dq.popleft()

                def drain():
                    if feed is not None:
                        while feed.pump():
                            pass
                    while dq:
                        try:
                            next(dq[0])
                        except StopIteration:
                            dq.popleft()

                return fill, drain

            # Phase A: b0h0 attention + b1 pair-2 loads + h0 qk + v spans 0,1
            feed2 = XFeed(1, 0, x_p2)
            fill_a, drain_a = make_fill([
                qk_pass(1, 0, x_p2, 0, 0, feed=feed2),
                qk_pass(1, 0, x_p2, 0, 1, feed=feed2),
                v_pass(1, 0, x_p2, 0, feed=feed2),
                v_pass(1, 1, x_p2, 1, feed=feed2),
            ], feed=feed2)
            attn_half(0, 0, 0, fill_a)
            attn_half(0, 0, 1, fill_a)
            drain_a()

            # Phase B: b0h1 attention + b1 pair-3 loads + h0 qk + v spans 2,3
            feed3 = XFeed(1, 1, x_p3)
            fill_b, drain_b = make_fill([
                qk_pass(1, 1, x_p3, 0, 0, feed=feed3),
                qk_pass(1, 1, x_p3, 0, 1, feed=feed3),
                v_pass(1, 2, x_p3, 0, feed=feed3),
                v_pass(1, 3, x_p3, 1, feed=feed3),
            ], feed=feed3)
            attn_half(0, 1, 0, fill_b)
            attn_half(0, 1, 1, fill_b)
            drain_b()

            # Phase C: b1h0 attention + b1h1 pair-0 qk passes
            fill_c, drain_c = make_fill([
                qk_pass(1, 0, x_p2, 1, 0),
                qk_pass(1, 0, x_p2, 1, 1),
            ])
            attn_half(1, 0, 0, fill_c)
            attn_half(1, 0, 1, fill_c)
            drain_c()

            # Phase D: b1h1 attention; its first half is filled with the
            # pair-1 qk passes (they only write spans 2,3 which half 0 never
            # reads), the second half borrows the then-idle qkv psum banks
            # for a deeper score pipeline.
            fill_d, drain_d = make_fill([
                qk_pass(1, 1, x_p3, 1, 0),
                qk_pass(1, 1, x_p3, 1, 1),
            ])
            attn_half(1, 1, 0, fill_d)
            drain_d()
            attn_half(1, 1, 1, lambda: None, extra_psc=True)

            if dbg:
                nc.sync.dma_start(qdump[:], q_sb[:])
                nc.sync.dma_start(kdump[:], k_sb[:])
                nc.sync.dma_start(vdump[:], v_sb[:])
    return nc


# ---------------------------------------------------------------------------
# NEFF 2: output projection, token-parallel
# ---------------------------------------------------------------------------

def build_oproj_nc():
    """out[hout, tok] = WoT.T @ attnT on a 2x4 (token-half x hout-quarter)
    core grid. Wo-stationary orientation: each LDWEIGHTS (wo k-tile x hout
    tile) feeds matmuls streaming 512-token chunks; the psum output
    [hout, tok] flushes to a host-transposed dram layout.

    dma_start issue costs ~0.6-1us on the issuing sequencer, so transfers
    are coarse and few: pass A covers token chunks 0,1 for all 4 hout tiles
    (8 psum banks) and only needs the tc01 HALF of each a-slab, so its feed
    is 256KB/kt against 1.7us/kt of matmul; the tc23 halves stream in bulk
    underneath and pass B (tc 2,3) runs entirely from SBUF.
    """
    nc = bass.Bass(target_bir_lowering=False, debug=False)

    TOKS = (B * S) // 2   # 2048 tokens per core (token half)
    HOUT = HIDDEN // 4    # 512 output channels per core (hout quarter)
    NHO = HOUT // 128     # 4 hout tiles
    NTC = TOKS // 512     # 4 token chunks
    attnT = nc.dram_tensor("attnT", [HIDDEN, TOKS], BF16, kind="ExternalInput")
    woT = nc.dram_tensor("woT", [HIDDEN, HOUT], BF16, kind="ExternalInput")
    out = nc.dram_tensor("out", [HOUT, TOKS], F32, kind="ExternalOutput")

    with tile.TileContext(nc) as tc:
        with (
            tc.tile_pool(name="persist", bufs=1) as persist,
            tc.tile_pool(name="outp", bufs=2) as outp,
            tc.tile_pool(name="psum", bufs=1, space="PSUM") as psum,
        ):
            a_bf = persist.tile([128, KT, TOKS], BF16, tag="a_bf")
            wo_bf = persist.tile([128, KT, HOUT], BF16, tag="wo_bf")

            def load_kt(kt):
                # a k-slab: [128, 2048] bf16 straight from dram (no cast).
                # The first two slabs load in 512-tok chunks so the first
                # matmul only waits on 128KB.
                if kt < 2:
                    for c in range(NTC):
                        nc.sync.dma_start(
                            a_bf[:, kt, c * 512:(c + 1) * 512],
                            attnT[kt * 128:(kt + 1) * 128,
                                  c * 512:(c + 1) * 512],
                        )
                else:
                    nc.sync.dma_start(
                        a_bf[:, kt, :], attnT[kt * 128:(kt + 1) * 128, :]
                    )

            def flush(ps_t, ho, tc_i, use_vec):
                o = outp.tile([128, 512], F32, tag=f"o{tc_i}")
                if use_vec:
                    nc.vector.tensor_copy(o[:], ps_t[:])
                else:
                    nc.scalar.copy(o[:], ps_t[:])
                nc.sync.dma_start(
                    out[ho * 128:(ho + 1) * 128, tc_i * 512:(tc_i + 1) * 512],
                    o[:],
                )

            # wo piece loads issue just-in-time from the (otherwise idle)
            # scalar sequencer: small 128KB transfers that never queue-jump
            # ahead of the bandwidth-critical a-slab stream.
            def load_wo(kt):
                nc.scalar.dma_start(
                    wo_bf[:, kt, :], woT[kt * 128:(kt + 1) * 128, :]
                )

            load_wo(0)
            PREFETCH = 3
            for kt in range(PREFETCH):
                load_kt(kt)
            load_wo(1)
            load_wo(2)

            # pass A (ho 0,1): kt-inner so matmuls chase the kt-slab DMAs;
            # all 8 psum banks accumulate across the full kt loop.
            ps_a = {(ho, tc_i): psum.tile([128, 512], F32, tag=f"ps{ho}{tc_i}",
                                          name=f"psa{ho}{tc_i}")
                    for ho in (0, 1) for tc_i in range(NTC)}
            for kt in range(KT):
                if kt + PREFETCH < KT:
                    load_kt(kt + PREFETCH)
                    load_wo(kt + PREFETCH)
                for ho in (0, 1):
                    for tc_i in range(NTC):
                        nc.tensor.matmul(
                            ps_a[(ho, tc_i)][:],
                            wo_bf[:, kt, ho * 128:(ho + 1) * 128],
                            a_bf[:, kt, tc_i * 512:(tc_i + 1) * 512],
                            start=(kt == 0),
                            stop=(kt == KT - 1),
                        )
            # flush pass A before pass B reuses the banks (WAR through Tile);
            # copies alternate Scalar/Vector and the DMAs ride under pass B.
            for i, (ho, tc_i) in enumerate([(h, t) for h in (0, 1)
                                            for t in range(NTC)]):
                flush(ps_a[(ho, tc_i)], ho, tc_i, i % 2 == 0)

            # pass B (ho 2,3): everything cached in SBUF; tc-outer so output
            # tiles complete staggered and flushes overlap remaining matmuls.
            # ho-outer within each tc so every psum group's stop staggers
            # and its flush overlaps the next group's matmuls
            for tc_i in range(NTC):
                for ho in (2, 3):
                    ps_b = psum.tile([128, 512], F32, tag=f"ps{ho - 2}{tc_i}",
                                     name=f"psb{ho}{tc_i}")
                    for kt in range(KT):
                        nc.tensor.matmul(
                            ps_b[:],
                            wo_bf[:, kt, ho * 128:(ho + 1) * 128],
                            a_bf[:, kt, tc_i * 512:(tc_i + 1) * 512],
                            start=(kt == 0),
                            stop=(kt == KT - 1),
                        )
                    flush(ps_b, ho, tc_i, ho == 2)
    return nc


# ---------------------------------------------------------------------------
# host driver
# ---------------------------------------------------------------------------

_NC_CACHE = {}


def _get_ncs():
    if "attn" not in _NC_CACHE:
        _NC_CACHE["attn"] = build_attn_nc()
        _NC_CACHE["oproj"] = build_oproj_nc()
    return _NC_CACHE["attn"], _NC_CACHE["oproj"]


def _rope_tables():
    inv_freq = 1.0 / (10000.0 ** (np.arange(0, HD, 2, dtype=np.float32) / HD))
    t = np.arange(S, dtype=np.float32)
    freqs = np.einsum("i,j->ij", t, inv_freq)  # [S, HD/2]
    emb = np.concatenate([freqs, freqs], axis=-1)  # [S, HD]
    cos = np.cos(emb).astype(np.float32)
    sin = np.sin(emb).astype(np.float32)
    cosT = np.ascontiguousarray(cos.T)  # [HD, S]
    sinT = np.ascontiguousarray(sin.T)
    sinT_signed = sinT.copy()
    sinT_signed[0:64, :] *= -1.0  # fold rotate_half's negation into the table
    return cosT.astype(NPBF16), sinT_signed.astype(NPBF16)


def kernel(x, Wq, Wk, Wv, Wo):
    x = np.asarray(x, dtype=np.float32)
    Wq = np.asarray(Wq, dtype=np.float32)
    Wk = np.asarray(Wk, dtype=np.float32)
    Wv = np.asarray(Wv, dtype=np.float32)
    Wo = np.asarray(Wo, dtype=np.float32)

    nc1, nc2 = _get_ncs()
    core_ids = list(range(N_CORES))
    trace = bool(os.environ.get("BASS_TRACE"))

    cosT, sinT_signed = _rope_tables()
    mask = np.triu(np.ones((128, 128), dtype=NPBF16))  # mask[k,q]=1 iff k<=q
    xT = np.ascontiguousarray(
        x.transpose(0, 2, 1)).astype(NPBF16)  # [B, HIDDEN, S] bf16

    in_maps1 = []
    for c in range(N_CORES):
        csl = slice(c * HPC * HD, (c + 1) * HPC * HD)
        in_maps1.append(
            {
                "xT": xT,
                "wqT": np.ascontiguousarray(Wq[csl, :].T).astype(NPBF16),
                "wkT": np.ascontiguousarray(Wk[csl, :].T).astype(NPBF16),
                "wvT": np.ascontiguousarray(Wv[csl, :].T).astype(NPBF16),
                "cosT": cosT,
                "sinT": sinT_signed,
                "mask": mask,
            }
        )

    LAST_RESULTS.clear()
    res1 = run_bass_kernel_spmd(nc1, in_maps1, core_ids=core_ids, trace=trace)
    LAST_RESULTS.append(res1)

    # host relayout: per-head attention outputs -> attnT [HIDDEN, B*S]
    arr = np.stack([res1.results[c]["attnout"] for c in range(N_CORES)])
    # axes: (core, h, b, span, q, c4, ch) -> d = core*256 + h*128 + ch,
    # tok = b*2048 + span*512 + c4*128 + q
    attnT = np.ascontiguousarray(
        arr.transpose(0, 1, 6, 2, 3, 5, 4).reshape(HIDDEN, B * S)
    )
    woT = np.ascontiguousarray(Wo.T).astype(NPBF16)

    TOKS = (B * S) // 2
    HOUT = HIDDEN // 4
    in_maps2 = []
    for c in range(N_CORES):
        ti, hj = c // 4, c % 4
        in_maps2.append(
            {
                "attnT": np.ascontiguousarray(attnT[:, ti * TOKS:(ti + 1) * TOKS]),
                "woT": np.ascontiguousarray(woT[:, hj * HOUT:(hj + 1) * HOUT]),
            }
        )
    res2 = run_bass_kernel_spmd(nc2, in_maps2, core_ids=core_ids, trace=trace)
    LAST_RESULTS.append(res2)

    out = np.empty((B * S, HIDDEN), dtype=np.float32)
    for c in range(N_CORES):
        ti, hj = c // 4, c % 4
        out[ti * TOKS:(ti + 1) * TOKS, hj * HOUT:(hj + 1) * HOUT] = (
            res2.results[c]["out"].T
        )
    return np.ascontiguousarray(out.reshape(B, S, HIDDEN), dtype=np.float32)
